# revision 3
# baseline (speedup 1.0000x reference)
"""SE(3) compose-scan Trainium2 kernel (nn_ComposeRt).

x [131072, 32, 3, 4] fp32 -> cumulative compose along axis 1:
out[b,0] = x[b,0]; out[b,n] = out[b,n-1] o x[b,n],
[rA|tA] o [rB|tB] = [rA@rB | tA + rA@tB].

Sharding: pure data parallel over batch across 8 NeuronCores.
Per core: batch b_local = t*(P*F) + p*F + f (mega-tile t, partition p,
slot f). DRAM I/O blocks [MEGA*HALVES, P, F*NSUB*12]; block (t, h) holds
n-range [h*NSUB, (h+1)*NSUB), SBUF layout [p][f][n][i*4+j]. The scan runs
sequentially over n with vector-engine tensor ops batched over (f, i, j)
via broadcast access patterns.
"""

import sys

if "/opt/trn_rl_repo" not in sys.path:
    sys.path.insert(0, "/opt/trn_rl_repo")

import numpy as np

import concourse.bass as bass
import concourse.bacc as bacc
import concourse.mybir as mybir
from concourse import bass_utils
from concourse.tile import TileContext

P = 128
N = 32
N_CORES = 8
B = 131072

# tunables
F = 32  # batch slots per partition per mega-tile
NSUB = 8  # n per sub-tile (DMA block)
MEGA = 4  # mega-tiles per core; MEGA*P*F == B // N_CORES
GP_TILES = ()  # mega-tile indices run on gpsimd instead of vector
HALVES = N // NSUB
B_CORE = B // N_CORES
assert MEGA * P * F == B_CORE


def _compose(nc, eng, C, A, Bm, tmp):
    """C = A o B. C/A/Bm/tmp: [P, F, 3, 4] views."""
    sh = list(C.shape)
    eng.tensor_mul(
        out=C,
        in0=A[:, :, :, 0:1].broadcast_to(sh),
        in1=Bm[:, :, 0:1, :].broadcast_to(sh),
    )
    eng.tensor_mul(
        out=tmp,
        in0=A[:, :, :, 1:2].broadcast_to(sh),
        in1=Bm[:, :, 1:2, :].broadcast_to(sh),
    )
    eng.tensor_add(out=C, in0=C, in1=tmp)
    eng.tensor_mul(
        out=tmp,
        in0=A[:, :, :, 2:3].broadcast_to(sh),
        in1=Bm[:, :, 2:3, :].broadcast_to(sh),
    )
    eng.tensor_add(out=C, in0=C, in1=tmp)
    eng.tensor_add(out=C[:, :, :, 3], in0=C[:, :, :, 3], in1=A[:, :, :, 3])


def _build():
    BLK = F * NSUB * 12
    nc = bacc.Bacc("TRN2", target_bir_lowering=False, debug=False)
    x = nc.dram_tensor(
        "x", [MEGA * HALVES, P, BLK], mybir.dt.float32, kind="ExternalInput"
    )
    y = nc.dram_tensor(
        "y", [MEGA * HALVES, P, BLK], mybir.dt.float32, kind="ExternalOutput"
    )

    with TileContext(nc) as tc:
        with (
            tc.tile_pool(name="xin", bufs=3) as xpool,
            tc.tile_pool(name="xin_gp", bufs=3) as xpool_gp,
            tc.tile_pool(name="outp", bufs=3) as opool,
            tc.tile_pool(name="outp_gp", bufs=3) as opool_gp,
            tc.tile_pool(name="tmps", bufs=2) as tpool,
        ):
            for t in range(MEGA):
                on_gp = t in GP_TILES
                eng = nc.gpsimd if on_gp else nc.vector
                xp = xpool_gp if on_gp else xpool
                op = opool_gp if on_gp else opool
                ttag = "tmp_gp" if on_gp else "tmp_v"
                prev = None
                for h in range(HALVES):
                    xt = xp.tile([P, BLK], mybir.dt.float32, tag="x")
                    nc.sync.dma_start(out=xt[:], in_=x.ap()[t * HALVES + h])
                    ot = op.tile([P, BLK], mybir.dt.float32, tag="o")
                    xv = xt.rearrange("p (f n i j) -> p f n i j", f=F, n=NSUB, i=3)
                    ov = ot.rearrange("p (f n i j) -> p f n i j", f=F, n=NSUB, i=3)
                    for nl in range(NSUB):
                        if h == 0 and nl == 0:
                            nc.scalar.copy(out=ov[:, :, 0], in_=xv[:, :, 0])
                            continue
                        A = ov[:, :, nl - 1] if nl > 0 else prev[:, :, NSUB - 1]
                        tmp = tpool.tile([P, F * 12], mybir.dt.float32, tag=ttag)
                        tv = tmp.rearrange("p (f i j) -> p f i j", f=F, i=3)
                        _compose(nc, eng, ov[:, :, nl], A, xv[:, :, nl], tv)
                    nc.sync.dma_start(out=y.ap()[t * HALVES + h], in_=ot[:])
                    prev = ov
    nc.compile()
    return nc


_NC_CACHE = []


def _get_nc():
    if not _NC_CACHE:
        _NC_CACHE.append(_build())
    return _NC_CACHE[0]


def _shard_input(x_full):
    out = []
    for c in range(N_CORES):
        xc = x_full[c * B_CORE : (c + 1) * B_CORE].reshape(MEGA, P, F, N, 12)
        xc = xc.reshape(MEGA, P, F, HALVES, NSUB, 12)
        xc = np.ascontiguousarray(xc.transpose(0, 3, 1, 2, 4, 5))
        out.append(xc.reshape(MEGA * HALVES, P, F * NSUB * 12))
    return out


def _unshard_output(ys):
    parts = []
    for yc in ys:
        a = yc.reshape(MEGA, HALVES, P, F, NSUB, 12)
        a = a.transpose(0, 2, 3, 1, 4, 5).reshape(B_CORE, N, 3, 4)
        parts.append(a)
    return np.concatenate(parts, axis=0)


def run(x, trace=False, trace_kwargs=None):
    """Returns (out [B,N,3,4], BassKernelResults)."""
    x = np.asarray(x, dtype=np.float32).reshape(B, N, 12)
    nc = _get_nc()
    in_maps = [{"x": xc} for xc in _shard_input(x)]
    res = bass_utils.run_bass_kernel_spmd(
        nc,
        in_maps,
        list(range(N_CORES)),
        trace=trace,
        **(trace_kwargs or {}),
    )
    out = _unshard_output([r["y"] for r in res.results])
    return out.reshape(B, N, 3, 4), res


def kernel(x):
    return run(x)[0]
